# revision 21
# baseline (speedup 1.0000x reference)
"""Causal self-attention (B=4, T=2048, C=1024, H=16) on 8 TRN2 NeuronCores.

Sharding: tensor-parallel over heads. Each core owns 2 heads:
  - computes its 384-column slice of the QKV projection (q|k|v, 128 cols each)
    directly in transposed layout qkvT = w_slice.T @ xT (x is host-pre-transposed),
  - runs causal attention for its 8 (batch, head) pairs in scores-transposed
    form sT = kT.T @ qT so that softmax(p) feeds the p@v matmul with no PE
    transposes; the softmax normalizer Z is accumulated by an appended
    ones-column in the v stationary operand,
  - normalizes y = yu * (1/Z) BEFORE the projection (1/Z broadcast to the
    128-partition dim via a tiny K=1 PE matmul), so the projection runs as a
    full K=128 contraction, and its PSUM output is copied to SBUF in bf16 and
    DMA'd out as a bf16 partial [8192, 1024].
Host sums the 8 partials in float64 and adds b_proj.

Causal structure: for query group g (512 queries), only key chunks 0..4g+3 are
computed; within the 4 diagonal chunks the score/exp/pv work is narrowed to the
columns at-or-below the diagonal and only the [128,128] triangle band is masked
(gpsimd affine_select in-place on the exp'd probabilities).

Software pipeline: QKV row-group n of batch b is emission-interleaved into
attention query-group g=n-1 of the same batch (and QKV(b+1, n=0) into
attention(b, g=3)), so the PE always has dense independent matmul work while
ScalarE runs exp.
"""

import numpy as np

import concourse.bass as bass
import concourse.mybir as mybir
import concourse.tile as tile
from concourse import bacc
from concourse.bass_utils import run_bass_kernel_spmd
from concourse.masks import make_identity

B, T, C, H, D = 4, 2048, 1024, 16, 64
NCORES = 8
HPC = H // NCORES          # heads per core = 2
SH = HPC * D               # 128: shard width of each of q/k/v
R = B * T                  # 8192 rows
KC = C // 128              # 8 contraction chunks
QG = T // 512              # 4 query groups per (batch, head) pair
CPB = T // 128             # 16 key chunks per batch
f32 = mybir.dt.float32
bf16 = mybir.dt.bfloat16
EXP = mybir.ActivationFunctionType.Exp

TRACE = False
TRACE_KWARGS = {}
LAST_RESULT = None
_NC_CACHE = None


def _emit(tc, xT, w_s, b_s, wp_s, out):
    nc = tc.nc

    with (
        tc.tile_pool(name="const", bufs=1) as constp,
        tc.tile_pool(name="qkv", bufs=2) as qkvp,
        tc.tile_pool(name="xt", bufs=2) as xtp,
        tc.tile_pool(name="vst", bufs=2) as vstp,
        tc.tile_pool(name="pt", bufs=6) as ptp,
        tc.tile_pool(name="yy", bufs=2) as yyp,
        tc.tile_pool(name="stg", bufs=3) as stgp,
        tc.tile_pool(name="ps_s", bufs=2, space="PSUM") as ps_s,
        tc.tile_pool(name="ps_o", bufs=2, space="PSUM") as ps_o,
        tc.tile_pool(name="aux", bufs=2, space="PSUM") as auxp,
    ):
        ident = constp.tile([128, 128], bf16)
        make_identity(nc, ident[:])
        # ebc row 64, cols 0:128   = h0 map: 1s in cols 0:64  -> partitions 0-63
        # ebc row 64, cols 128:256 = h1 map: 1s in cols 64:128 -> partitions 64-127
        ebc = constp.tile([65, 256], bf16)
        nc.vector.memset(ebc[:], 0.0)
        nc.vector.memset(ebc[64:65, 0:64], 1.0)
        nc.vector.memset(ebc[64:65, 192:256], 1.0)
        w_sb = constp.tile([128, KC, 3 * SH], bf16)
        nc.sync.dma_start(w_sb[:], w_s.rearrange("(kc p) m -> p kc m", p=128))
        b_sb = constp.tile([128, 3], f32)
        nc.sync.dma_start(b_sb[:], b_s.rearrange("(m p) -> p m", p=128))
        wp_sb = constp.tile([128, C], bf16)
        nc.sync.dma_start(wp_sb[:], wp_s)

        xTv = xT.rearrange("(kc p) m -> p kc m", p=128)

        # per-batch persistent tiles
        st = {}

        def qkv_units(b):
            """QKV projection for batch b: 4 row-groups, each a flat list of
            single-matmul-granularity micro-units for fine interleaving."""
            qT = qkvp.tile([128, T], bf16, name=f"qT_{b}", tag="qT")
            kT = qkvp.tile([128, T], bf16, name=f"kT_{b}", tag="kT")
            # vaug block (h, key-chunk): [d0..d63 | ones]
            vaug = qkvp.tile([128, 2 * CPB * 65], bf16, name=f"vaug_{b}", tag="vaug")
            yu = yyp.tile([128, T], bf16, name=f"yu_{b}", tag="yu")
            ysc = yyp.tile([128, T], bf16, name=f"ysc_{b}", tag="ysc")
            # 1/Z rows live at partition 64 (same partition as the ot Z rows)
            RB0 = yyp.tile([65, T], bf16, name=f"RB0_{b}", tag="rb0")
            RB1 = yyp.tile([65, T], bf16, name=f"RB1_{b}", tag="rb1")
            st[b] = (qT, kT, vaug, yu, ysc, RB0, RB1)
            vblk = vaug[:].rearrange("p (blk c) -> p blk c", c=65)

            groups = []
            for n in range(QG):
                nsl = slice(512 * n, 512 * (n + 1))
                gnsl = slice(512 * (4 * b + n), 512 * (4 * b + n + 1))

                units = []

                def u_dma(nsl=nsl, gnsl=gnsl, n=n):
                    xt = xtp.tile([128, KC, 512], bf16, name=f"xt_{b}_{n}", tag="xt")
                    nc.sync.dma_start(xt[:], xTv[:, :, gnsl])
                    st[(b, n)] = xt
                    # ones columns for this row-group's 4 key chunks (both heads)
                    nc.gpsimd.memset(vblk[:, 4 * n : 4 * n + 4, 64:65], 1.0)
                    nc.gpsimd.memset(vblk[:, CPB + 4 * n : CPB + 4 * n + 4, 64:65], 1.0)

                units.append(u_dma)

                def u_mm1(m, k, n=n):
                    def unit():
                        if k == 0:
                            st[(b, n, "ps")] = auxp.tile(
                                [128, 512], f32, name=f"ps_{b}_{n}_{m}", tag="aux"
                            )
                        ps = st[(b, n, "ps")]
                        nc.tensor.matmul(
                            ps[:],
                            w_sb[:, k, 128 * m : 128 * (m + 1)],
                            st[(b, n)][:, k, :],
                            start=(k == 0),
                            stop=(k == KC - 1),
                        )

                    return unit

                def u_copy(m, nsl=nsl, n=n):
                    def unit():
                        ps = st[(b, n, "ps")]
                        if m == 0:
                            nc.vector.tensor_scalar_add(qT[:, nsl], ps[:], b_sb[:, 0:1])
                        elif m == 1:
                            nc.vector.tensor_scalar_add(kT[:, nsl], ps[:], b_sb[:, 1:2])
                        else:
                            vst = vstp.tile(
                                [128, 512], bf16, name=f"vst_{b}_{n}", tag="vst"
                            )
                            nc.vector.tensor_scalar_add(vst[:], ps[:], b_sb[:, 2:3])
                            st[(b, n, "v")] = vst

                    return unit

                for m in range(3):
                    for k in range(KC):
                        units.append(u_mm1(m, k))
                    units.append(u_copy(m))

                def u_v(j, n=n):
                    def unit():
                        vst = st[(b, n, "v")]
                        cl = 4 * n + j
                        tp = auxp.tile(
                            [128, 128], bf16, name=f"tp_{b}_{n}_{j}", tag="aux"
                        )
                        nc.tensor.transpose(
                            tp[:], vst[:, 128 * j : 128 * (j + 1)], ident[:]
                        )
                        off0 = cl * 65
                        off1 = (CPB + cl) * 65
                        nc.vector.tensor_copy(vaug[:, off0 : off0 + 64], tp[:, 0:64])
                        nc.vector.tensor_copy(vaug[:, off1 : off1 + 64], tp[:, 64:128])

                    return unit

                for j in range(4):
                    units.append(u_v(j))
                groups.append(units)
            return groups

        def att_units(b, g, pull):
            """Attention query-group g of batch b + finalize + projection.
            `pull(k)` emits up to k filler micro-units at stall-join points."""
            qT, kT, vaug, yu, ysc, RB0, RB1 = st[b]
            nkc = 4 * g + 4
            ots = [
                ps_o.tile([65, 512], f32, name=f"ot_{b}_{h}_{g}", tag="ot")
                for h in range(2)
            ]
            gsl = slice(512 * g, 512 * (g + 1))
            units = []

            for kc2 in range(nkc // 2):
                def step(kc2=kc2):
                    sps = [
                        ps_s.tile([128, 1024], f32, name=f"sp_{b}_{h}_{g}_{kc2}", tag="sp")
                        for h in range(2)
                    ]
                    pts = [
                        ptp.tile([128, 1024], bf16, name=f"pt_{b}_{h}_{g}_{kc2}", tag="pt")
                        for h in range(2)
                    ]
                    los = []
                    for half in range(2):
                        kc = 2 * kc2 + half
                        j = kc - 4 * g
                        lo = 128 * j if j > 0 else 0
                        los.append(lo)
                        for h in range(2):
                            hsl = slice(64 * h, 64 * h + 64)
                            nc.tensor.matmul(
                                sps[h][:, 512 * half + lo : 512 * (half + 1)],
                                kT[hsl, 128 * kc : 128 * (kc + 1)],
                                qT[hsl, 512 * g + lo : 512 * (g + 1)],
                                start=True,
                                stop=True,
                            )
                    pull(2)
                    for h in range(2):
                        nc.scalar.activation(
                            pts[h][:, los[0] : 1024],
                            sps[h][:, los[0] : 1024],
                            EXP,
                            scale=0.125,
                        )
                    pull(1)
                    for half in range(2):
                        kc = 2 * kc2 + half
                        j = kc - 4 * g
                        if j >= 0:
                            # triangle band: keep col f >= partition p
                            c0 = 512 * half + 128 * j
                            for h in range(2):
                                nc.gpsimd.affine_select(
                                    out=pts[h][:, c0 : c0 + 128],
                                    in_=pts[h][:, c0 : c0 + 128],
                                    compare_op=mybir.AluOpType.is_ge,
                                    fill=0.0,
                                    base=0,
                                    channel_multiplier=-1,
                                    pattern=[[1, 128]],
                                )
                    pull(1)
                    for half in range(2):
                        kc = 2 * kc2 + half
                        lo = los[half]
                        for h in range(2):
                            voff = (h * CPB + kc) * 65
                            nc.tensor.matmul(
                                ots[h][:, lo:512],
                                vaug[:, voff : voff + 65],
                                pts[h][:, 512 * half + lo : 512 * (half + 1)],
                                start=(kc == 0),
                                stop=(kc == nkc - 1),
                                skip_group_check=True,
                            )
                    pull(2)

                units.append(step)

            def fin():
                # ot_h: y dims at partitions 0-63, Z at partition 64.
                # Recips first (they gate ysc -> proj), then the yu copies.
                # Full-tile approx recip (partition-parallel, ~free-dim cost);
                # only row 64 (the Z row) is meaningful, rows 0-63 discarded.
                zr = stgp.tile([65, 1024], f32, name=f"zr_{b}_{g}", tag="zr")
                nc.vector.reciprocal_approx_fast(zr[:, 0:512], ots[0][:])
                nc.vector.reciprocal_approx_fast(zr[:, 512:1024], ots[1][:])
                with nc.allow_low_precision("1/Z rows rounded to bf16"):
                    nc.vector.tensor_copy(RB0[64:65, gsl], zr[64:65, 0:512])
                    nc.vector.tensor_copy(RB1[64:65, gsl], zr[64:65, 512:1024])
                rbc = ps_o.tile([128, 512], f32, name=f"rbc_{b}_{g}", tag="ot")
                nc.tensor.matmul(
                    rbc[:], ebc[64:65, 0:128], RB0[64:65, gsl], start=True, stop=False
                )
                nc.tensor.matmul(
                    rbc[:], ebc[64:65, 128:256], RB1[64:65, gsl], start=False, stop=True
                )
                nc.vector.tensor_copy(yu[0:64, gsl], ots[0][0:64, :])
                stage = stgp.tile([64, 512], bf16, name=f"stage_{b}_{g}", tag="stage")
                nc.vector.tensor_copy(stage[:], ots[1][0:64, :])
                nc.sync.dma_start(yu[64:128, gsl], stage[:])
                pull(2)
                nc.vector.tensor_mul(ysc[:, gsl], yu[:, gsl], rbc[:])

            units.append(fin)

            for rt in range(4 * g, 4 * (g + 1)):
                def proj(rt=rt):
                    rsl = slice(128 * rt, 128 * (rt + 1))
                    r0 = b * T + 128 * rt
                    ost = stgp.tile([128, 1024], bf16, name=f"ost_{b}_{rt}", tag="ost")
                    for jn in range(2):
                        nsl = slice(512 * jn, 512 * (jn + 1))
                        pout = ps_o.tile(
                            [128, 512], f32, name=f"po_{b}_{rt}_{jn}", tag="ot"
                        )
                        nc.tensor.matmul(
                            pout[:], ysc[:, rsl], wp_sb[:, nsl], start=True, stop=True
                        )
                        nc.vector.tensor_copy(ost[:, nsl], pout[:])
                        pull(1)
                    nc.sync.dma_start(out[r0 : r0 + 128, :], ost[:])

                units.append(proj)

            return units

        # ---- schedule: QKV(b, n) fills attention(b, g=n-1) at single-matmul
        # granularity via pull() callbacks at the stall-join points ----
        qgroups = [qkv_units(b) for b in range(B)]
        filler = []
        fstate = {"i": 0, "budget": 0.0, "per": 0.0}

        def pull(k):
            n = min(len(filler) - fstate["i"], k)
            for _ in range(max(0, n)):
                filler[fstate["i"]]()
                fstate["i"] += 1

        for u in qgroups[0][0]:
            u()
        for b in range(B):
            for g in range(QG):
                if g < QG - 1:
                    newf = qgroups[b][g + 1]
                elif b + 1 < B:
                    newf = qgroups[b + 1][0]
                else:
                    newf = []
                filler = filler[fstate["i"] :] + newf
                fstate["i"] = 0
                au = att_units(b, g, pull)
                for u in au:
                    u()
            # batch boundary: drain any leftover filler
            pull(len(filler))


def build_nc():
    global _NC_CACHE
    if _NC_CACHE is not None:
        return _NC_CACHE
    nc = bacc.Bacc("TRN2", target_bir_lowering=False, debug=False)
    xT = nc.dram_tensor("xT", [C, R], bf16, kind="ExternalInput").ap()
    w_s = nc.dram_tensor("w_s", [C, 3 * SH], bf16, kind="ExternalInput").ap()
    b_s = nc.dram_tensor("b_s", [3 * SH], f32, kind="ExternalInput").ap()
    wp_s = nc.dram_tensor("wp_s", [SH, C], bf16, kind="ExternalInput").ap()
    out = nc.dram_tensor("out", [R, C], bf16, kind="ExternalOutput").ap()
    with tile.TileContext(nc) as tc:
        _emit(tc, xT, w_s, b_s, wp_s, out)
    nc.compile()
    _NC_CACHE = nc
    return nc


def kernel(x, w_attn, b_attn, w_proj, b_proj):
    global LAST_RESULT
    x = np.asarray(x, dtype=np.float32)
    w_attn = np.asarray(w_attn, dtype=np.float32)
    b_attn = np.asarray(b_attn, dtype=np.float32)
    w_proj = np.asarray(w_proj, dtype=np.float32)
    b_proj = np.asarray(b_proj, dtype=np.float32)

    import ml_dtypes

    xTh = np.ascontiguousarray(x.reshape(R, C).T.astype(ml_dtypes.bfloat16))  # [C, R]
    in_maps = []
    for c in range(NCORES):
        csl = slice(SH * c, SH * (c + 1))
        w_slice = np.ascontiguousarray(
            np.concatenate(
                [w_attn[:, csl], w_attn[:, C:][:, csl], w_attn[:, 2 * C :][:, csl]],
                axis=1,
            )
        )
        b_slice = np.ascontiguousarray(
            np.concatenate([b_attn[csl], b_attn[C:][csl], b_attn[2 * C :][csl]])
        )
        wp_slice = np.ascontiguousarray(w_proj[csl, :].astype(ml_dtypes.bfloat16))
        w_slice = w_slice.astype(ml_dtypes.bfloat16)
        in_maps.append({"xT": xTh, "w_s": w_slice, "b_s": b_slice, "wp_s": wp_slice})

    nc = build_nc()
    res = run_bass_kernel_spmd(
        nc,
        in_maps,
        core_ids=list(range(NCORES)),
        trace=TRACE,
        **TRACE_KWARGS,
    )
    LAST_RESULT = res
    acc = np.zeros((R, C), dtype=np.float64)
    for c in range(NCORES):
        acc += res.results[c]["out"].astype(np.float64)
    out = (acc + b_proj.astype(np.float64)).astype(np.float32)
    return out.reshape(B, T, C)


# revision 24
# speedup vs baseline: 1.0086x; 1.0086x over previous
"""Causal self-attention (B=4, T=2048, C=1024, H=16) on 8 TRN2 NeuronCores.

Sharding: tensor-parallel over heads. Each core owns 2 heads:
  - computes its 384-column slice of the QKV projection (q|k|v, 128 cols each)
    directly in transposed layout qkvT = w_slice.T @ xT (x is host-pre-transposed),
  - runs causal attention for its 8 (batch, head) pairs in scores-transposed
    form sT = kT.T @ qT so that softmax(p) feeds the p@v matmul with no PE
    transposes; the softmax normalizer Z is accumulated by an appended
    ones-column in the v stationary operand,
  - normalizes y = yu * (1/Z) BEFORE the projection (1/Z broadcast to the
    128-partition dim via a tiny K=1 PE matmul), so the projection runs as a
    full K=128 contraction, and its PSUM output is copied to SBUF in bf16 and
    DMA'd out as a bf16 partial [8192, 1024].
Host sums the 8 partials in float64 and adds b_proj.

Causal structure: for query group g (512 queries), only key chunks 0..4g+3 are
computed; within the 4 diagonal chunks the score/exp/pv work is narrowed to the
columns at-or-below the diagonal and only the [128,128] triangle band is masked
(gpsimd affine_select in-place on the exp'd probabilities).

Software pipeline: QKV row-group n of batch b is emission-interleaved into
attention query-group g=n-1 of the same batch (and QKV(b+1, n=0) into
attention(b, g=3)), so the PE always has dense independent matmul work while
ScalarE runs exp.
"""

import numpy as np

import concourse.bass as bass
import concourse.mybir as mybir
import concourse.tile as tile
from concourse import bacc
from concourse.bass_utils import run_bass_kernel_spmd
from concourse.masks import make_identity

B, T, C, H, D = 4, 2048, 1024, 16, 64
NCORES = 8
HPC = H // NCORES          # heads per core = 2
SH = HPC * D               # 128: shard width of each of q/k/v
R = B * T                  # 8192 rows
KC = C // 128              # 8 contraction chunks
QG = T // 512              # 4 query groups per (batch, head) pair
CPB = T // 128             # 16 key chunks per batch
f32 = mybir.dt.float32
bf16 = mybir.dt.bfloat16
EXP = mybir.ActivationFunctionType.Exp

TRACE = False
TRACE_KWARGS = {}
LAST_RESULT = None
_NC_CACHE = None


def _emit(tc, xT, w_s, b_s, wp_s, out):
    nc = tc.nc

    with (
        tc.tile_pool(name="const", bufs=1) as constp,
        tc.tile_pool(name="qkv", bufs=2) as qkvp,
        tc.tile_pool(name="xt", bufs=2) as xtp,
        tc.tile_pool(name="vst", bufs=2) as vstp,
        tc.tile_pool(name="pt", bufs=6) as ptp,
        tc.tile_pool(name="yy", bufs=2) as yyp,
        tc.tile_pool(name="stg", bufs=3) as stgp,
        tc.tile_pool(name="ps_s", bufs=2, space="PSUM") as ps_s,
        tc.tile_pool(name="ps_o", bufs=2, space="PSUM") as ps_o,
        tc.tile_pool(name="aux", bufs=2, space="PSUM") as auxp,
    ):
        ident = constp.tile([128, 128], bf16)
        make_identity(nc, ident[:])
        # ebc row 64, cols 0:128   = h0 map: 1s in cols 0:64  -> partitions 0-63
        # ebc row 64, cols 128:256 = h1 map: 1s in cols 64:128 -> partitions 64-127
        ebc = constp.tile([65, 256], bf16)
        nc.vector.memset(ebc[:], 0.0)
        nc.vector.memset(ebc[64:65, 0:64], 1.0)
        nc.vector.memset(ebc[64:65, 192:256], 1.0)
        w_sb = constp.tile([128, KC, 3 * SH], bf16)
        nc.sync.dma_start(w_sb[:], w_s.rearrange("(kc p) m -> p kc m", p=128))
        b_sb = constp.tile([128, 3], f32)
        nc.sync.dma_start(b_sb[:], b_s.rearrange("(m p) -> p m", p=128))
        wp_sb = constp.tile([128, C], bf16)
        nc.sync.dma_start(wp_sb[:], wp_s)

        xTv = xT.rearrange("(kc p) m -> p kc m", p=128)

        # per-batch persistent tiles
        st = {}

        def qkv_units(b):
            """QKV projection for batch b: 4 row-groups, each a flat list of
            single-matmul-granularity micro-units for fine interleaving."""
            qT = qkvp.tile([128, T], bf16, name=f"qT_{b}", tag="qT")
            kT = qkvp.tile([128, T], bf16, name=f"kT_{b}", tag="kT")
            # vaug block (h, key-chunk): [d0..d63 | ones]
            vaug = qkvp.tile([128, 2 * CPB * 65], bf16, name=f"vaug_{b}", tag="vaug")
            yu = yyp.tile([128, T], bf16, name=f"yu_{b}", tag="yu")
            ysc = yyp.tile([128, T], bf16, name=f"ysc_{b}", tag="ysc")
            # 1/Z rows live at partition 64 (same partition as the ot Z rows)
            RB0 = yyp.tile([65, T], bf16, name=f"RB0_{b}", tag="rb0")
            RB1 = yyp.tile([65, T], bf16, name=f"RB1_{b}", tag="rb1")
            st[b] = (qT, kT, vaug, yu, ysc, RB0, RB1)
            vblk = vaug[:].rearrange("p (blk c) -> p blk c", c=65)

            groups = []
            for n in range(QG):
                nsl = slice(512 * n, 512 * (n + 1))
                gnsl = slice(512 * (4 * b + n), 512 * (4 * b + n + 1))

                units = []

                def u_dma(nsl=nsl, gnsl=gnsl, n=n):
                    xt = xtp.tile([128, KC, 512], bf16, name=f"xt_{b}_{n}", tag="xt")
                    nc.sync.dma_start(xt[:], xTv[:, :, gnsl])
                    st[(b, n)] = xt
                    # ones columns for this row-group's 4 key chunks (both heads)
                    nc.gpsimd.memset(vblk[:, 4 * n : 4 * n + 4, 64:65], 1.0)
                    nc.gpsimd.memset(vblk[:, CPB + 4 * n : CPB + 4 * n + 4, 64:65], 1.0)

                units.append(u_dma)

                def u_mm1(m, k, n=n):
                    def unit():
                        if k == 0:
                            st[(b, n, "ps")] = auxp.tile(
                                [128, 512], f32, name=f"ps_{b}_{n}_{m}", tag="aux"
                            )
                        ps = st[(b, n, "ps")]
                        nc.tensor.matmul(
                            ps[:],
                            w_sb[:, k, 128 * m : 128 * (m + 1)],
                            st[(b, n)][:, k, :],
                            start=(k == 0),
                            stop=(k == KC - 1),
                        )

                    return unit

                def u_copy(m, nsl=nsl, n=n):
                    def unit():
                        ps = st[(b, n, "ps")]
                        if m == 0:
                            nc.vector.tensor_scalar_add(qT[:, nsl], ps[:], b_sb[:, 0:1])
                        elif m == 1:
                            nc.vector.tensor_scalar_add(kT[:, nsl], ps[:], b_sb[:, 1:2])
                        else:
                            vst = vstp.tile(
                                [128, 512], bf16, name=f"vst_{b}_{n}", tag="vst"
                            )
                            nc.vector.tensor_scalar_add(vst[:], ps[:], b_sb[:, 2:3])
                            st[(b, n, "v")] = vst

                    return unit

                for m in range(3):
                    for k in range(KC):
                        units.append(u_mm1(m, k))
                    units.append(u_copy(m))

                def u_v(j, n=n):
                    def unit():
                        vst = st[(b, n, "v")]
                        cl = 4 * n + j
                        tp = auxp.tile(
                            [128, 128], bf16, name=f"tp_{b}_{n}_{j}", tag="aux"
                        )
                        nc.tensor.transpose(
                            tp[:], vst[:, 128 * j : 128 * (j + 1)], ident[:]
                        )
                        off0 = cl * 65
                        off1 = (CPB + cl) * 65
                        nc.vector.tensor_copy(vaug[:, off0 : off0 + 64], tp[:, 0:64])
                        nc.vector.tensor_copy(vaug[:, off1 : off1 + 64], tp[:, 64:128])

                    return unit

                for j in range(4):
                    units.append(u_v(j))
                groups.append(units)
            return groups

        def att_units(b, g, pull):
            """Attention query-group g of batch b + finalize + projection.
            `pull(k)` emits up to k filler micro-units at stall-join points."""
            qT, kT, vaug, yu, ysc, RB0, RB1 = st[b]
            nkc = 4 * g + 4
            ots = [
                ps_o.tile([65, 512], f32, name=f"ot_{b}_{h}_{g}", tag="ot")
                for h in range(2)
            ]
            gsl = slice(512 * g, 512 * (g + 1))
            units = []

            for kc2 in range(nkc // 2):
                def step(kc2=kc2):
                    sps = [
                        ps_s.tile([128, 1024], f32, name=f"sp_{b}_{h}_{g}_{kc2}", tag="sp")
                        for h in range(2)
                    ]
                    pts = [
                        ptp.tile([128, 1024], bf16, name=f"pt_{b}_{h}_{g}_{kc2}", tag="pt")
                        for h in range(2)
                    ]
                    los = []
                    for half in range(2):
                        kc = 2 * kc2 + half
                        j = kc - 4 * g
                        lo = 128 * j if j > 0 else 0
                        los.append(lo)
                        for h in range(2):
                            hsl = slice(64 * h, 64 * h + 64)
                            nc.tensor.matmul(
                                sps[h][:, 512 * half + lo : 512 * (half + 1)],
                                kT[hsl, 128 * kc : 128 * (kc + 1)],
                                qT[hsl, 512 * g + lo : 512 * (g + 1)],
                                start=True,
                                stop=True,
                            )
                    for h in range(2):
                        nc.scalar.activation(
                            pts[h][:, los[0] : 1024],
                            sps[h][:, los[0] : 1024],
                            EXP,
                            scale=0.125,
                        )
                    for half in range(2):
                        kc = 2 * kc2 + half
                        j = kc - 4 * g
                        if j >= 0:
                            # triangle band: keep col f >= partition p
                            c0 = 512 * half + 128 * j
                            for h in range(2):
                                nc.gpsimd.affine_select(
                                    out=pts[h][:, c0 : c0 + 128],
                                    in_=pts[h][:, c0 : c0 + 128],
                                    compare_op=mybir.AluOpType.is_ge,
                                    fill=0.0,
                                    base=0,
                                    channel_multiplier=-1,
                                    pattern=[[1, 128]],
                                )
                    for half in range(2):
                        kc = 2 * kc2 + half
                        lo = los[half]
                        for h in range(2):
                            voff = (h * CPB + kc) * 65
                            nc.tensor.matmul(
                                ots[h][:, lo:512],
                                vaug[:, voff : voff + 65],
                                pts[h][:, 512 * half + lo : 512 * (half + 1)],
                                start=(kc == 0),
                                stop=(kc == nkc - 1),
                                skip_group_check=True,
                            )
                    pull(6)

                units.append(step)

            def fin():
                # ot_h: y dims at partitions 0-63, Z at partition 64.
                # Recips first (they gate ysc -> proj), then the yu copies.
                # Full-tile approx recip (partition-parallel, ~free-dim cost);
                # only row 64 (the Z row) is meaningful, rows 0-63 discarded.
                zr = stgp.tile([65, 1024], f32, name=f"zr_{b}_{g}", tag="zr")
                nc.vector.reciprocal_approx_fast(zr[:, 0:512], ots[0][:])
                nc.vector.reciprocal_approx_fast(zr[:, 512:1024], ots[1][:])
                with nc.allow_low_precision("1/Z rows rounded to bf16"):
                    nc.vector.tensor_copy(RB0[64:65, gsl], zr[64:65, 0:512])
                    nc.vector.tensor_copy(RB1[64:65, gsl], zr[64:65, 512:1024])
                rbc = ps_o.tile([128, 512], f32, name=f"rbc_{b}_{g}", tag="ot")
                nc.tensor.matmul(
                    rbc[:], ebc[64:65, 0:128], RB0[64:65, gsl], start=True, stop=False
                )
                nc.tensor.matmul(
                    rbc[:], ebc[64:65, 128:256], RB1[64:65, gsl], start=False, stop=True
                )
                nc.vector.tensor_copy(yu[0:64, gsl], ots[0][0:64, :])
                stage = stgp.tile([64, 512], bf16, name=f"stage_{b}_{g}", tag="stage")
                nc.vector.tensor_copy(stage[:], ots[1][0:64, :])
                nc.sync.dma_start(yu[64:128, gsl], stage[:])
                pull(2)
                nc.vector.tensor_mul(ysc[:, gsl], yu[:, gsl], rbc[:])

            units.append(fin)

            for rt in range(4 * g, 4 * (g + 1)):
                def proj(rt=rt):
                    rsl = slice(128 * rt, 128 * (rt + 1))
                    r0 = b * T + 128 * rt
                    ost = stgp.tile([128, 1024], bf16, name=f"ost_{b}_{rt}", tag="ost")
                    for jn in range(2):
                        nsl = slice(512 * jn, 512 * (jn + 1))
                        pout = ps_o.tile(
                            [128, 512], f32, name=f"po_{b}_{rt}_{jn}", tag="ot"
                        )
                        nc.tensor.matmul(
                            pout[:], ysc[:, rsl], wp_sb[:, nsl], start=True, stop=True
                        )
                        nc.vector.tensor_copy(ost[:, nsl], pout[:])
                        pull(1)
                    nc.sync.dma_start(out[r0 : r0 + 128, :], ost[:])

                units.append(proj)

            return units

        # ---- schedule: QKV(b, n) fills attention(b, g=n-1) at single-matmul
        # granularity via pull() callbacks at the stall-join points ----
        qgroups = [qkv_units(b) for b in range(B)]
        filler = []
        fstate = {"i": 0, "budget": 0.0, "per": 0.0}

        def pull(k):
            n = min(len(filler) - fstate["i"], k)
            for _ in range(max(0, n)):
                filler[fstate["i"]]()
                fstate["i"] += 1

        for u in qgroups[0][0]:
            u()
        for b in range(B):
            for g in range(QG):
                if g < QG - 1:
                    newf = qgroups[b][g + 1]
                elif b + 1 < B:
                    newf = qgroups[b + 1][0]
                else:
                    newf = []
                filler = filler[fstate["i"] :] + newf
                fstate["i"] = 0
                au = att_units(b, g, pull)
                for u in au:
                    u()
            # batch boundary: drain any leftover filler
            pull(len(filler))


def build_nc():
    global _NC_CACHE
    if _NC_CACHE is not None:
        return _NC_CACHE
    nc = bacc.Bacc("TRN2", target_bir_lowering=False, debug=False)
    xT = nc.dram_tensor("xT", [C, R], bf16, kind="ExternalInput").ap()
    w_s = nc.dram_tensor("w_s", [C, 3 * SH], bf16, kind="ExternalInput").ap()
    b_s = nc.dram_tensor("b_s", [3 * SH], f32, kind="ExternalInput").ap()
    wp_s = nc.dram_tensor("wp_s", [SH, C], bf16, kind="ExternalInput").ap()
    out = nc.dram_tensor("out", [R, C], bf16, kind="ExternalOutput").ap()
    with tile.TileContext(nc) as tc:
        _emit(tc, xT, w_s, b_s, wp_s, out)
    nc.compile()
    _NC_CACHE = nc
    return nc


def kernel(x, w_attn, b_attn, w_proj, b_proj):
    global LAST_RESULT
    x = np.asarray(x, dtype=np.float32)
    w_attn = np.asarray(w_attn, dtype=np.float32)
    b_attn = np.asarray(b_attn, dtype=np.float32)
    w_proj = np.asarray(w_proj, dtype=np.float32)
    b_proj = np.asarray(b_proj, dtype=np.float32)

    import ml_dtypes

    xTh = np.ascontiguousarray(x.reshape(R, C).T.astype(ml_dtypes.bfloat16))  # [C, R]
    in_maps = []
    for c in range(NCORES):
        csl = slice(SH * c, SH * (c + 1))
        w_slice = np.ascontiguousarray(
            np.concatenate(
                [w_attn[:, csl], w_attn[:, C:][:, csl], w_attn[:, 2 * C :][:, csl]],
                axis=1,
            )
        )
        b_slice = np.ascontiguousarray(
            np.concatenate([b_attn[csl], b_attn[C:][csl], b_attn[2 * C :][csl]])
        )
        wp_slice = np.ascontiguousarray(w_proj[csl, :].astype(ml_dtypes.bfloat16))
        w_slice = w_slice.astype(ml_dtypes.bfloat16)
        in_maps.append({"xT": xTh, "w_s": w_slice, "b_s": b_slice, "wp_s": wp_slice})

    nc = build_nc()
    res = run_bass_kernel_spmd(
        nc,
        in_maps,
        core_ids=list(range(NCORES)),
        trace=TRACE,
        **TRACE_KWARGS,
    )
    LAST_RESULT = res
    acc = np.zeros((R, C), dtype=np.float64)
    for c in range(NCORES):
        acc += res.results[c]["out"].astype(np.float64)
    out = (acc + b_proj.astype(np.float64)).astype(np.float32)
    return out.reshape(B, T, C)


# revision 26
# speedup vs baseline: 1.1224x; 1.1128x over previous
"""Causal self-attention (B=4, T=2048, C=1024, H=16) on 8 TRN2 NeuronCores.

Sharding: 2-way batch x 4-way head. Core c owns batches {2bg, 2bg+1}
(bg = c//4) and heads 4hg..4hg+4 (hg = c%4):
  - computes its 768-column slice of the QKV projection (q|k|v, 256 cols each)
    in transposed layout qkvT = w_slice.T @ xT for its two batches (x is
    host-pre-transposed and batch-sliced),
  - runs causal attention for its 8 (batch, head) pairs as two head-pairs per
    batch, in scores-transposed form sT = kT.T @ qT so softmax(p) feeds the
    p@v matmul with no PE transposes; the softmax normalizer Z is accumulated
    by an appended ones-column in the v stationary operand,
  - normalizes y = yu * (1/Z) BEFORE the projection (1/Z broadcast to the
    128-partition dim via a tiny K=1 PE matmul), so the projection runs as a
    K=256 contraction (2 accumulating K=128 matmuls), and its PSUM output is
    copied to SBUF bf16 and DMA'd out as a bf16 partial [4096, 1024].
Host sums the 4 head-group partials per batch-pair in float64, adds b_proj.

Causal structure: for query group g (512 queries), only key chunks 0..4g+3 are
computed; within the 4 diagonal chunks the score/exp/pv work is narrowed to the
columns at-or-below the diagonal and only the [128,128] triangle band is masked
(gpsimd affine_select in-place on the exp'd probabilities).

Software pipeline: QKV row-group n of batch b is emission-interleaved into
attention query-group g=n-1 of the same batch (and QKV(b+1, n=0) into
attention(b, g=3)), so the PE always has dense independent matmul work while
ScalarE runs exp.
"""

import numpy as np

import concourse.bass as bass
import concourse.mybir as mybir
import concourse.tile as tile
from concourse import bacc
from concourse.bass_utils import run_bass_kernel_spmd
from concourse.masks import make_identity

B, T, C, H, D = 4, 2048, 1024, 16, 64
NCORES = 8
BPC = 2                    # batches per core
HPC = 4                    # heads per core
HP = HPC // 2              # head-pairs per core = 2
SH = HPC * D               # 256: shard width of each of q/k/v
R2 = BPC * T               # 4096 rows per core
KC = C // 128              # 8 contraction chunks
QG = T // 512              # 4 query groups per (batch, head) pair
CPB = T // 128             # 16 key chunks per batch
f32 = mybir.dt.float32
bf16 = mybir.dt.bfloat16
EXP = mybir.ActivationFunctionType.Exp

TRACE = False
TRACE_KWARGS = {}
LAST_RESULT = None
_NC_CACHE = None


def _emit(tc, xT, w_s, b_s, wp_s, out):
    nc = tc.nc

    with (
        tc.tile_pool(name="const", bufs=1) as constp,
        tc.tile_pool(name="qkv", bufs=2) as qkvp,
        tc.tile_pool(name="xt", bufs=2) as xtp,
        tc.tile_pool(name="vst", bufs=2) as vstp,
        tc.tile_pool(name="pt", bufs=6) as ptp,
        tc.tile_pool(name="yy", bufs=2) as yyp,
        tc.tile_pool(name="stg", bufs=3) as stgp,
        tc.tile_pool(name="ps_s", bufs=2, space="PSUM") as ps_s,
        tc.tile_pool(name="ps_o", bufs=2, space="PSUM") as ps_o,
        tc.tile_pool(name="aux", bufs=2, space="PSUM") as auxp,
    ):
        ident = constp.tile([128, 128], bf16)
        make_identity(nc, ident[:])
        # ebc row 64, cols 0:128   = intra-pair h0 map: 1s in cols 0:64
        # ebc row 64, cols 128:256 = intra-pair h1 map: 1s in cols 64:128
        ebc = constp.tile([65, 256], bf16)
        nc.vector.memset(ebc[:], 0.0)
        nc.vector.memset(ebc[64:65, 0:64], 1.0)
        nc.vector.memset(ebc[64:65, 192:256], 1.0)
        w_sb = constp.tile([128, KC, 3 * SH], bf16)
        nc.sync.dma_start(w_sb[:], w_s.rearrange("(kc p) m -> p kc m", p=128))
        b_sb = constp.tile([128, 6], f32)
        nc.sync.dma_start(b_sb[:], b_s.rearrange("(m p) -> p m", p=128))
        wp_sb = constp.tile([128, HP, C], bf16)
        nc.sync.dma_start(wp_sb[:], wp_s.rearrange("(hp p) m -> p hp m", p=128))

        xTv = xT.rearrange("(kc p) m -> p kc m", p=128)

        # per-batch state: st[b] = dict with per-head-pair tiles
        st = {}

        def qkv_units(b):
            """QKV projection for local batch b as 4 row-groups x 9 units."""
            s = {}
            for hp in range(HP):
                s[("qT", hp)] = qkvp.tile([128, T], bf16, name=f"qT_{b}_{hp}", tag=f"qT{hp}")
                s[("kT", hp)] = qkvp.tile([128, T], bf16, name=f"kT_{b}_{hp}", tag=f"kT{hp}")
                # vaug block (h, key-chunk): [d0..d63 | ones]
                s[("vaug", hp)] = qkvp.tile(
                    [128, 2 * CPB * 65], bf16, name=f"va_{b}_{hp}", tag=f"va{hp}"
                )
                s[("yu", hp)] = yyp.tile([128, T], bf16, name=f"yu_{b}_{hp}", tag=f"yu{hp}")
                s[("ysc", hp)] = yyp.tile([128, T], bf16, name=f"ys_{b}_{hp}", tag=f"ys{hp}")
                # 1/Z rows live at partition 64 (same partition as the ot Z rows)
                s[("RB0", hp)] = yyp.tile([65, T], bf16, name=f"R0_{b}_{hp}", tag=f"r0{hp}")
                s[("RB1", hp)] = yyp.tile([65, T], bf16, name=f"R1_{b}_{hp}", tag=f"r1{hp}")
            st[b] = s

            # m-chunk -> (dest kind, head-pair): q0 q1 k0 k1 v0 v1
            groups = []
            for n in range(QG):
                nsl = slice(512 * n, 512 * (n + 1))
                gnsl = slice(512 * (4 * b + n), 512 * (4 * b + n + 1))

                def u_dma(gnsl=gnsl, n=n):
                    xt = xtp.tile([128, KC, 512], bf16, name=f"xt_{b}_{n}", tag="xt")
                    nc.sync.dma_start(xt[:], xTv[:, :, gnsl])
                    st[(b, n)] = xt

                def u_mm(m, nsl=nsl, n=n):
                    def unit():
                        xt = st[(b, n)]
                        ps = auxp.tile(
                            [128, 512], f32, name=f"ps_{b}_{n}_{m}", tag="aux"
                        )
                        for k in range(KC):
                            nc.tensor.matmul(
                                ps[:],
                                w_sb[:, k, 128 * m : 128 * (m + 1)],
                                xt[:, k, :],
                                start=(k == 0),
                                stop=(k == KC - 1),
                            )
                        hp = m % 2
                        if m < 2:
                            nc.vector.tensor_scalar_add(
                                s[("qT", hp)][:, nsl], ps[:], b_sb[:, m : m + 1]
                            )
                        elif m < 4:
                            nc.vector.tensor_scalar_add(
                                s[("kT", hp)][:, nsl], ps[:], b_sb[:, m : m + 1]
                            )
                        else:
                            vst = vstp.tile(
                                [128, 512], bf16, name=f"vs_{b}_{n}_{hp}", tag=f"vst{hp}"
                            )
                            nc.vector.tensor_scalar_add(vst[:], ps[:], b_sb[:, m : m + 1])
                            st[(b, n, "v", hp)] = vst

                    return unit

                def u_v(hp, n=n):
                    def unit():
                        vst = st[(b, n, "v", hp)]
                        vaug = s[("vaug", hp)]
                        vblk = vaug[:].rearrange("p (blk c) -> p blk c", c=65)
                        # ones columns for this row-group's 4 key chunks
                        nc.gpsimd.memset(vblk[:, 4 * n : 4 * n + 4, 64:65], 1.0)
                        nc.gpsimd.memset(
                            vblk[:, CPB + 4 * n : CPB + 4 * n + 4, 64:65], 1.0
                        )
                        for j in range(4):
                            cl = 4 * n + j
                            tp = auxp.tile(
                                [128, 128], bf16, name=f"tp_{b}_{n}_{hp}_{j}", tag="aux"
                            )
                            nc.tensor.transpose(
                                tp[:], vst[:, 128 * j : 128 * (j + 1)], ident[:]
                            )
                            off0 = cl * 65
                            off1 = (CPB + cl) * 65
                            nc.vector.tensor_copy(vaug[:, off0 : off0 + 64], tp[:, 0:64])
                            nc.vector.tensor_copy(
                                vaug[:, off1 : off1 + 64], tp[:, 64:128]
                            )

                    return unit

                groups.append(
                    [u_dma]
                    + [u_mm(m) for m in range(6)]
                    + [u_v(0), u_v(1)]
                )
            return groups

        def att_units(b, g):
            """Attention query-group g of batch b (both head-pairs) + proj."""
            s = st[b]
            nkc = 4 * g + 4
            gsl = slice(512 * g, 512 * (g + 1))
            units = []

            def mk_step(hp, kc2):
                def step():
                    qT, kT, vaug = s[("qT", hp)], s[("kT", hp)], s[("vaug", hp)]
                    if kc2 == 0:
                        s[(g, hp, "ots")] = [
                            ps_o.tile(
                                [65, 512], f32, name=f"ot_{b}_{g}_{hp}_{h}", tag="ot"
                            )
                            for h in range(2)
                        ]
                    ots = s[(g, hp, "ots")]
                    sps = [
                        ps_s.tile(
                            [128, 1024], f32, name=f"sp_{b}_{g}_{hp}_{kc2}_{h}", tag="sp"
                        )
                        for h in range(2)
                    ]
                    pts = [
                        ptp.tile(
                            [128, 1024], bf16, name=f"pt_{b}_{g}_{hp}_{kc2}_{h}", tag="pt"
                        )
                        for h in range(2)
                    ]
                    los = []
                    for half in range(2):
                        kc = 2 * kc2 + half
                        j = kc - 4 * g
                        lo = 128 * j if j > 0 else 0
                        los.append(lo)
                        for h in range(2):
                            hsl = slice(64 * h, 64 * h + 64)
                            nc.tensor.matmul(
                                sps[h][:, 512 * half + lo : 512 * (half + 1)],
                                kT[hsl, 128 * kc : 128 * (kc + 1)],
                                qT[hsl, 512 * g + lo : 512 * (g + 1)],
                                start=True,
                                stop=True,
                            )
                    for h in range(2):
                        nc.scalar.activation(
                            pts[h][:, los[0] : 1024],
                            sps[h][:, los[0] : 1024],
                            EXP,
                            scale=0.125,
                        )
                    for half in range(2):
                        kc = 2 * kc2 + half
                        j = kc - 4 * g
                        if j >= 0:
                            # triangle band: keep col f >= partition p
                            c0 = 512 * half + 128 * j
                            for h in range(2):
                                nc.gpsimd.affine_select(
                                    out=pts[h][:, c0 : c0 + 128],
                                    in_=pts[h][:, c0 : c0 + 128],
                                    compare_op=mybir.AluOpType.is_ge,
                                    fill=0.0,
                                    base=0,
                                    channel_multiplier=-1,
                                    pattern=[[1, 128]],
                                )
                    for half in range(2):
                        kc = 2 * kc2 + half
                        lo = los[half]
                        for h in range(2):
                            voff = (h * CPB + kc) * 65
                            nc.tensor.matmul(
                                ots[h][:, lo:512],
                                vaug[:, voff : voff + 65],
                                pts[h][:, 512 * half + lo : 512 * (half + 1)],
                                start=(kc == 0),
                                stop=(kc == nkc - 1),
                                skip_group_check=True,
                            )

                return step

            def mk_fin(hp):
                def fin():
                    ots = s[(g, hp, "ots")]
                    yu, ysc = s[("yu", hp)], s[("ysc", hp)]
                    RB0, RB1 = s[("RB0", hp)], s[("RB1", hp)]
                    # ot_h: y dims at partitions 0-63, Z at partition 64
                    nc.vector.tensor_copy(yu[0:64, gsl], ots[0][0:64, :])
                    stage = stgp.tile(
                        [64, 512], bf16, name=f"st_{b}_{g}_{hp}", tag="stage"
                    )
                    nc.vector.tensor_copy(stage[:], ots[1][0:64, :])
                    nc.sync.dma_start(yu[64:128, gsl], stage[:])
                    # full-tile approx recip (partition-parallel, ~free-dim
                    # cost); only row 64 (the Z row) is meaningful
                    zr = stgp.tile([65, 1024], f32, name=f"zr_{b}_{g}_{hp}", tag="zr")
                    nc.vector.reciprocal_approx_fast(zr[:, 0:512], ots[0][:])
                    nc.vector.reciprocal_approx_fast(zr[:, 512:1024], ots[1][:])
                    with nc.allow_low_precision("1/Z rows rounded to bf16"):
                        nc.vector.tensor_copy(RB0[64:65, gsl], zr[64:65, 0:512])
                        nc.vector.tensor_copy(RB1[64:65, gsl], zr[64:65, 512:1024])
                    rbc = ps_o.tile([128, 512], f32, name=f"rbc_{b}_{g}_{hp}", tag="ot")
                    nc.tensor.matmul(
                        rbc[:], ebc[64:65, 0:128], RB0[64:65, gsl],
                        start=True, stop=False,
                    )
                    nc.tensor.matmul(
                        rbc[:], ebc[64:65, 128:256], RB1[64:65, gsl],
                        start=False, stop=True,
                    )
                    nc.vector.tensor_mul(ysc[:, gsl], yu[:, gsl], rbc[:])

                return fin

            for hp in range(HP):
                for kc2 in range(nkc // 2):
                    units.append(mk_step(hp, kc2))
                units.append(mk_fin(hp))

            for rt in range(4 * g, 4 * (g + 1)):
                def proj(rt=rt):
                    rsl = slice(128 * rt, 128 * (rt + 1))
                    r0 = b * T + 128 * rt
                    ost = stgp.tile([128, 1024], bf16, name=f"os_{b}_{rt}", tag="ost")
                    for jn in range(2):
                        nsl = slice(512 * jn, 512 * (jn + 1))
                        pout = ps_o.tile(
                            [128, 512], f32, name=f"po_{b}_{rt}_{jn}", tag="ot"
                        )
                        for hp in range(HP):
                            nc.tensor.matmul(
                                pout[:],
                                s[("ysc", hp)][:, rsl],
                                wp_sb[:, hp, nsl],
                                start=(hp == 0),
                                stop=(hp == HP - 1),
                            )
                        nc.vector.tensor_copy(ost[:, nsl], pout[:])
                    nc.sync.dma_start(out[r0 : r0 + 128, :], ost[:])

                units.append(proj)

            return units

        # ---- schedule: QKV(b, n) fills attention(b, g=n-1) ----
        qgroups = [qkv_units(b) for b in range(BPC)]
        for u in qgroups[0][0]:
            u()
        for b in range(BPC):
            for g in range(QG):
                au = att_units(b, g)
                if g < QG - 1:
                    filler = qgroups[b][g + 1]
                elif b + 1 < BPC:
                    filler = qgroups[b + 1][0]
                else:
                    filler = []
                stride = max(1, len(au) // (len(filler) + 1))
                fi = 0
                for i, u in enumerate(au):
                    u()
                    if fi < len(filler) and (i + 1) % stride == 0:
                        filler[fi]()
                        fi += 1
                while fi < len(filler):
                    filler[fi]()
                    fi += 1


def build_nc():
    global _NC_CACHE
    if _NC_CACHE is not None:
        return _NC_CACHE
    nc = bacc.Bacc("TRN2", target_bir_lowering=False, debug=False)
    xT = nc.dram_tensor("xT", [C, R2], bf16, kind="ExternalInput").ap()
    w_s = nc.dram_tensor("w_s", [C, 3 * SH], bf16, kind="ExternalInput").ap()
    b_s = nc.dram_tensor("b_s", [3 * SH], f32, kind="ExternalInput").ap()
    wp_s = nc.dram_tensor("wp_s", [SH, C], bf16, kind="ExternalInput").ap()
    out = nc.dram_tensor("out", [R2, C], bf16, kind="ExternalOutput").ap()
    with tile.TileContext(nc) as tc:
        _emit(tc, xT, w_s, b_s, wp_s, out)
    nc.compile()
    _NC_CACHE = nc
    return nc


def kernel(x, w_attn, b_attn, w_proj, b_proj):
    global LAST_RESULT
    x = np.asarray(x, dtype=np.float32)
    w_attn = np.asarray(w_attn, dtype=np.float32)
    b_attn = np.asarray(b_attn, dtype=np.float32)
    w_proj = np.asarray(w_proj, dtype=np.float32)
    b_proj = np.asarray(b_proj, dtype=np.float32)

    import ml_dtypes

    in_maps = []
    for c in range(NCORES):
        bg, hg = c // 4, c % 4
        xs = x[2 * bg : 2 * bg + 2].reshape(R2, C)
        xTh = np.ascontiguousarray(xs.T.astype(ml_dtypes.bfloat16))  # [C, R2]
        csl = slice(SH * hg, SH * (hg + 1))
        # m-chunk layout: q(hp0|hp1) k(hp0|hp1) v(hp0|hp1) -> just q|k|v slices
        w_slice = np.ascontiguousarray(
            np.concatenate(
                [w_attn[:, csl], w_attn[:, C:][:, csl], w_attn[:, 2 * C :][:, csl]],
                axis=1,
            )
        ).astype(ml_dtypes.bfloat16)
        b_slice = np.ascontiguousarray(
            np.concatenate([b_attn[csl], b_attn[C:][csl], b_attn[2 * C :][csl]])
        )
        wp_slice = np.ascontiguousarray(w_proj[csl, :].astype(ml_dtypes.bfloat16))
        in_maps.append({"xT": xTh, "w_s": w_slice, "b_s": b_slice, "wp_s": wp_slice})

    nc = build_nc()
    res = run_bass_kernel_spmd(
        nc,
        in_maps,
        core_ids=list(range(NCORES)),
        trace=TRACE,
        **TRACE_KWARGS,
    )
    LAST_RESULT = res
    acc = np.zeros((B * T, C), dtype=np.float64)
    for c in range(NCORES):
        bg = c // 4
        acc[2 * bg * T : (2 * bg + 2) * T] += res.results[c]["out"].astype(np.float64)
    out = (acc + b_proj.astype(np.float64)).astype(np.float32)
    return out.reshape(B, T, C)
